# revision 4
# baseline (speedup 1.0000x reference)
# Trainium2 Bass kernel for dense soft-MoE (nn_MANN_78726750536045).
#
# Math (per sample b):
#   gates = softmax(MLP_elu(x_gate))                     [K=8]
#   h0 = elu(sum_k g_k * (x_main @ W1_k.T) + gates@eb1)  [512]
#   h1 = elu(sum_k g_k * (h0 @ W2_k.T) + gates@eb2)      [512]
#   out =     sum_k g_k * (h1 @ W3_k.T) + gates@eb3      [512]
#
# Structure (v2, "output gating"):
#   Per expert k: Y_k = HT @ W_k  as 4 chunk matmuls (contraction over the
#   512 input features, 128 per chunk), accumulated in one PSUM bank.
#   Then t_k = (g_k * 2^-8) * Y_k via a per-partition scaled copy
#   (ScalarE / GpSimd alternating) and acc += t_k on VectorE.  The bias
#   term gates@eb seeds the accumulation from its own tiny matmul.
#   This removes the per-expert diag-build matmuls of v1: the tensor
#   stream is 32 main matmuls + 1 bias + 4 transposes per layer.
#
# Weights are stored in fp8 e3m4 (scaled by 2^8 host-side so the uniform
# init range lands in e3m4's normal range; the 2^-8 descale rides the
# existing per-partition gate scale).  This halves HBM traffic vs bf16:
# ~6.3 MB/core, ~19 us at the ~360 GB/s per-core share.  Activations are
# fp16 (more mantissa than bf16, range is tiny here).
#
# Sharding: pure data-parallel, batch 1024 -> 128 rows per core x 8 cores.
#
# A few dummy matmuls on zeros run during the initial DMA window to flip
# the PE HAM clock gate to 2.4 GHz before the real stream starts.

import numpy as np
import ml_dtypes

B = 1024
X_MAIN, X_GATE, HID, Y_DIM, GHID, K = 480, 128, 512, 512, 32, 8
NCORES = 8
BL = B // NCORES  # 128 rows per core
P = 128
NCH = 32  # contraction chunks per expert layer (K experts x 4 feature blocks)
WSCALE = 256.0  # weight scale (power of 2); descale folded into gate scale

# fp32 small-constant pack: column layout
_C_XGT = 0          # [128, 128] x_gate^T slice
_C_IDF = 128        # [128, 128] identity
_C_G1T = 256        # [128, 32]
_C_G2T = 288        # [32, 32]
_C_G3T = 320        # [32, 8]
_C_GB1 = 328        # [32, 1]
_C_GB2 = 329        # [32, 1]
_C_GB3 = 330        # [8, 1]
_SMF_W = 336

_cache = {}


def _build_nc():
    from contextlib import ExitStack

    import concourse.bacc as bacc
    import concourse.mybir as mybir
    import concourse.tile as tile

    f32 = mybir.dt.float32
    f16 = mybir.dt.float16
    e3 = mybir.dt.float8e3
    AF = mybir.ActivationFunctionType
    OP = mybir.AluOpType

    nc = bacc.Bacc("TRN2", target_bir_lowering=False, debug=False)

    # ---- DRAM I/O ----
    d_smf = nc.dram_tensor("smf", [P, _SMF_W], f32, kind="ExternalInput")
    d_smb = nc.dram_tensor("smb", [P, HID], f16, kind="ExternalInput")  # xmT
    d_be = nc.dram_tensor("be", [K, 3 * HID], f16, kind="ExternalInput")
    # weights, chunk c = k*4+j: w[p, c*512+o] = ew[k][o, j*128+p] * 256
    d_w = [
        nc.dram_tensor(f"w{l}", [P, NCH * HID], e3, kind="ExternalInput")
        for l in range(3)
    ]
    d_out = nc.dram_tensor("out", [BL, Y_DIM], f32, kind="ExternalOutput")

    with ExitStack() as ctx:
        tc = ctx.enter_context(tile.TileContext(nc))
        consts = ctx.enter_context(tc.tile_pool(name="consts", bufs=1))
        sb = ctx.enter_context(tc.tile_pool(name="sb", bufs=3))
        htp = ctx.enter_context(tc.tile_pool(name="htp", bufs=2))
        tkp = ctx.enter_context(tc.tile_pool(name="tkp", bufs=4))
        accp = ctx.enter_context(tc.tile_pool(name="accp", bufs=4))
        pY = ctx.enter_context(tc.tile_pool(name="pY", bufs=3, space="PSUM"))
        pB = ctx.enter_context(tc.tile_pool(name="pB", bufs=2, space="PSUM"))
        paux = ctx.enter_context(tc.tile_pool(name="paux", bufs=2, space="PSUM"))

        # ---- DMA: constants + weight pieces ----
        # scalar (ACT HWDGE) ring: small constants, then first half of w0
        smf = consts.tile([P, _SMF_W], f32)
        nc.scalar.dma_start(smf, d_smf[:])
        smb = consts.tile([P, HID], f16)
        nc.scalar.dma_start(smb, d_smb[:])
        be = consts.tile([K, 3 * HID], f16)
        nc.scalar.dma_start(be, d_be[:])

        t_w = []
        for l in range(3):
            wt = consts.tile([P, NCH * HID], e3, name=f"wl{l}")
            t_w.append(wt)
        HALF = NCH * HID // 2
        nc.scalar.dma_start(t_w[0][:, 0:HALF], d_w[0][:, 0:HALF])
        nc.scalar.dma_start(t_w[0][:, HALF:], d_w[0][:, HALF:])
        # sync (SP HWDGE) ring: w1 and w2, in consumption order
        nc.sync.dma_start(t_w[1][:, 0:HALF], d_w[1][:, 0:HALF])
        nc.sync.dma_start(t_w[1][:, HALF:], d_w[1][:, HALF:])
        nc.sync.dma_start(t_w[2][:, 0:HALF], d_w[2][:, 0:HALF])
        nc.sync.dma_start(t_w[2][:, HALF:], d_w[2][:, HALF:])

        t_xgT = smf[:, _C_XGT : _C_XGT + 128]
        t_idf = smf[:, _C_IDF : _C_IDF + 128]
        t_g1T = smf[:, _C_G1T : _C_G1T + GHID]
        t_g2T = smf[0:GHID, _C_G2T : _C_G2T + GHID]
        t_g3T = smf[0:GHID, _C_G3T : _C_G3T + K]
        t_gb1 = smf[0:GHID, _C_GB1 : _C_GB1 + 1]
        t_gb2 = smf[0:GHID, _C_GB2 : _C_GB2 + 1]
        t_gb3 = smf[0:K, _C_GB3 : _C_GB3 + 1]

        # ---- PE warmup: dummy matmuls on zeros flip HAM to full clock ----
        wz = consts.tile([P, HID], f16)
        nc.gpsimd.memset(wz, 0.0)
        for _ in range(8):
            pz = paux.tile([P, HID], f32, tag="paux")
            nc.tensor.matmul(pz, wz[:, 0:P], wz, start=True, stop=True)

        # ---- gating network (fp32, [feature, batch] layout) ----
        def elu_block(p_in, bias, width):
            e = sb.tile([width, BL], f32, tag="gelu_e")
            nc.scalar.activation(e, p_in, AF.Exp, bias=bias)
            r = sb.tile([width, BL], f32, tag="gelu_r")
            nc.vector.tensor_scalar(r, p_in, bias, 0.0, OP.add, OP.max)
            t = sb.tile([width, BL], f32, tag="gelu_t")
            nc.vector.tensor_scalar(t, e, -1.0, 0.0, OP.add, OP.min)
            g = sb.tile([width, BL], f32, tag="gelu_g")
            nc.vector.tensor_tensor(g, r, t, OP.add)
            return g

        p1 = paux.tile([GHID, BL], f32, tag="paux")
        nc.tensor.matmul(p1, t_g1T, t_xgT, start=True, stop=True)
        g1 = elu_block(p1, t_gb1, GHID)

        p2 = paux.tile([GHID, BL], f32, tag="paux")
        nc.tensor.matmul(p2, t_g2T, g1, start=True, stop=True)
        g2 = elu_block(p2, t_gb2, GHID)

        p3 = paux.tile([K, BL], f32, tag="paux")
        nc.tensor.matmul(p3, t_g3T, g2, start=True, stop=True)

        # softmax over K (partition dim): exp -> transpose [K,BL]->[BL,K]
        # -> free-dim sum + reciprocal + scale.
        es = sb.tile([K, BL], f32)
        nc.scalar.activation(es, p3, AF.Exp, bias=t_gb3)
        p_esT = paux.tile([BL, K], f32, tag="paux")
        nc.tensor.transpose(p_esT, es, t_idf[0:K, 0:K])
        ssum = sb.tile([BL, 1], f32)
        nc.vector.tensor_reduce(ssum, p_esT, mybir.AxisListType.X, OP.add)
        recip = sb.tile([BL, 1], f32)
        nc.vector.reciprocal(recip, ssum)
        recip2 = sb.tile([BL, 1], f32)
        nc.vector.tensor_scalar(recip2, recip, 1.0 / WSCALE, None, OP.mult)
        gates = sb.tile([BL, K], f32)  # true scale, for the bias path
        nc.vector.tensor_scalar(gates, p_esT, recip, None, OP.mult)
        gs = sb.tile([BL, K], f32)  # gate * 2^-8, for the Y_k descale
        nc.vector.tensor_scalar(gs, p_esT, recip2, None, OP.mult)

        # gates^T [K, BL] in fp16 (bias matmul lhsT)
        p_gT = paux.tile([K, BL], f32, tag="paux")
        nc.tensor.transpose(p_gT, gates, t_idf)
        gT = sb.tile([K, BL], f16)
        nc.vector.tensor_copy(gT, p_gT)

        # ---- three expert layers ----
        hT = smb  # [128(part)=feature-in-block, 512] fp16, block j at cols j*128
        for l in range(3):
            pm_b = pB.tile([P, HID], f32, tag="pb")
            nc.tensor.matmul(pm_b, gT, be[:, l * HID : (l + 1) * HID],
                             start=True, stop=True)
            acc = pm_b  # running accumulator (psum for k=0, then sbuf tiles)
            for k in range(K):
                pm = pY.tile([P, HID], f32, tag="py")
                for j in range(4):
                    c = k * 4 + j
                    nc.tensor.matmul(
                        pm,
                        hT[:, j * P : (j + 1) * P],
                        t_w[l][:, c * HID : (c + 1) * HID],
                        start=(j == 0),
                        stop=(j == 3),
                    )
                if k < K - 1:
                    t = tkp.tile([P, HID], f32, tag="tk")
                    if k % 2 == 0:
                        nc.scalar.activation(t, pm, AF.Copy, scale=gs[:, k : k + 1])
                    else:
                        nc.vector.tensor_scalar(t, pm, gs[:, k : k + 1], None, OP.mult)
                    nacc = accp.tile([P, HID], f32, tag="acc")
                    nc.vector.tensor_tensor(nacc, acc, t, OP.add)
                    acc = nacc
                else:
                    # last expert: striped tail (128-col stripes) so the
                    # ELU/transpose/output pipeline starts immediately
                    nacc = accp.tile([P, HID], f32, tag="acc")
                    t = tkp.tile([P, HID], f32, tag="tk")
                    for j in range(4):
                        cs = slice(j * P, (j + 1) * P)
                        if j % 2 == 0:
                            nc.scalar.activation(
                                t[:, cs], pm[:, cs], AF.Copy, scale=gs[:, k : k + 1]
                            )
                        else:
                            nc.vector.tensor_scalar(
                                t[:, cs], pm[:, cs], gs[:, k : k + 1], None, OP.mult
                            )
                        nc.vector.tensor_tensor(nacc[:, cs], acc[:, cs], t[:, cs],
                                                OP.add)
                    acc = nacc

            if l < 2:
                # ELU + transpose, per 128-col block:
                # elu(x) = max(x, min(exp(x)-1, 0))
                h32 = accp.tile([P, HID], f32, tag="h32")
                hT2 = htp.tile([P, HID], f16, tag="ht")
                for j in range(4):
                    cs = slice(j * P, (j + 1) * P)
                    e = sb.tile([P, P], f32, tag="ee")
                    nc.scalar.activation(e, acc[:, cs], AF.Exp)
                    tt = sb.tile([P, P], f32, tag="et")
                    nc.vector.tensor_scalar(tt, e, -1.0, 0.0, OP.add, OP.min)
                    nc.vector.tensor_tensor(h32[:, cs], acc[:, cs], tt, OP.max)
                    ptr = paux.tile([P, P], f32, tag="paux")
                    nc.tensor.transpose(ptr, h32[:, cs], t_idf)
                    nc.vector.tensor_copy(hT2[:, cs], ptr)
                hT = hT2
            else:
                nc.sync.dma_start(d_out[:, 0:256], acc[:, 0:256])
                nc.scalar.dma_start(d_out[:, 256:512], acc[:, 256:512])

    nc.compile()
    return nc


def _prep_inputs(inputs):
    f16 = np.float16
    e3m4 = ml_dtypes.float8_e3m4
    xm = np.asarray(inputs["x_main"], np.float32)
    xg = np.asarray(inputs["x_gate"], np.float32)

    xgT = np.ascontiguousarray(xg.T)  # [128, B]

    # fp32 small pack (per-core: xgT slice differs)
    smf_base = np.zeros((P, _SMF_W), np.float32)
    smf_base[:, _C_IDF : _C_IDF + 128] = np.eye(P, dtype=np.float32)
    smf_base[:, _C_G1T : _C_G1T + GHID] = np.asarray(inputs["gw1"], np.float32).T
    smf_base[0:GHID, _C_G2T : _C_G2T + GHID] = np.asarray(
        inputs["gw2"], np.float32
    ).T
    smf_base[0:GHID, _C_G3T : _C_G3T + K] = np.asarray(inputs["gw3"], np.float32).T
    smf_base[0:GHID, _C_GB1] = np.asarray(inputs["gb1"], np.float32)
    smf_base[0:GHID, _C_GB2] = np.asarray(inputs["gb2"], np.float32)
    smf_base[0:K, _C_GB3] = np.asarray(inputs["gb3"], np.float32)

    # expert biases [K, 3*512] fp16
    be = np.zeros((K, 3 * HID), f16)
    for l in range(3):
        be[:, l * HID : (l + 1) * HID] = np.asarray(
            inputs[f"eb{l + 1}"], np.float32
        ).astype(f16)

    # expert weights -> expert-major chunk layout, e3m4 * 256:
    # w[p, (k*4+j)*512 + o] = ew[k][o, j*128+p] * 256
    def pack_w(ew):
        ewt = np.asarray(ew, np.float32).transpose(0, 2, 1)  # [K, in, out]
        if ewt.shape[1] < HID:
            pad = np.zeros((K, HID, ewt.shape[2]), np.float32)
            pad[:, : ewt.shape[1], :] = ewt
            ewt = pad
        # dims (k, j, p, o) -> (p, k, j, o) -> [128, 8*4*512]
        w = ewt.reshape(K, 4, P, HID).transpose(2, 0, 1, 3).reshape(P, NCH * HID)
        return np.ascontiguousarray((w * WSCALE).astype(e3m4))

    w = [pack_w(inputs["ew1"]), pack_w(inputs["ew2"]), pack_w(inputs["ew3"])]

    # xmT per core: [128(part)=feature-within-block, 4*128] fp16
    # smb[p, j*128+b] = x_main[row0+b, j*128+p]
    xmp = np.zeros((B, HID), np.float32)
    xmp[:, :X_MAIN] = xm

    in_maps = []
    for i in range(NCORES):
        smf = smf_base.copy()
        smf[:, _C_XGT : _C_XGT + 128] = xgT[:, i * BL : (i + 1) * BL]
        blk = xmp[i * BL : (i + 1) * BL]  # [128 b, 512 f]
        smb = np.ascontiguousarray(
            blk.reshape(BL, 4, P).transpose(2, 1, 0).reshape(P, HID).astype(f16)
        )
        m = {
            "smf": smf,
            "smb": smb,
            "be": be,
            "w0": w[0],
            "w1": w[1],
            "w2": w[2],
        }
        in_maps.append(m)
    return in_maps


def kernel(**inputs):
    from concourse.bass_utils import run_bass_kernel_spmd

    if "nc" not in _cache:
        _cache["nc"] = _build_nc()
    nc = _cache["nc"]

    in_maps = _prep_inputs(inputs)
    res = run_bass_kernel_spmd(nc, in_maps, core_ids=list(range(NCORES)))
    out = np.concatenate([r["out"] for r in res.results], axis=0)
    return np.ascontiguousarray(out.astype(np.float32))
